# revision 18
# baseline (speedup 1.0000x reference)
"""BiLSTM-CRF Trainium2 kernel (8 NeuronCores, data-parallel over batch).

Layout strategy per core (B_local=8, S=256, E=256, H=256, T=32):
 - Embedding gather on device via indirect DMA, then PE-transpose to
   xT [E-slab(128), (t, b)].
 - Input projection Xp = x @ Wih.T + b hoisted as one big matmul in
   transposed orientation (M = 4H tiles), staged to DRAM, streamed back
   per 64-step chunk.
 - Recurrence keeps state transposed: c,h in [128(u), (hi,dir,b)] so the
   per-step matmul out=z (M=b) uses h slices directly as lhsT; z is
   PE-transposed back (row-group packed) into gate layout.
 - Both directions interleaved as independent chains (fwd t ascending,
   bwd t descending).
 - feats = h @ Wout.T + bout on device; Viterbi DP + backtrace on host.
Gate order remapped to (i, f, o, g) so sigmoid covers a contiguous block.
"""

import numpy as np

VOCAB, TAGS, EDIM, HDIM = 50000, 32, 256, 512
HALF = HDIM // 2
NEG = -10000.0
B, S = 64, 256
NCORES = 8
BL = B // NCORES  # 8 examples per core
F32 = None  # set after mybir import

_CACHE = {}


def _orig4h(col):
    # col = gate_new*256 + hi*128 + u  with new gate order (i,f,o,g)
    gmap = [0, 1, 3, 2]  # new -> orig quarter
    g, r = divmod(col, 256)
    return gmap[g] * 256 + r


def _build_bass():
    import concourse.bass as bass
    import concourse.tile as tile
    from concourse import bacc, mybir

    f32 = mybir.dt.float32
    i32 = mybir.dt.int32
    AF = mybir.ActivationFunctionType
    OP = mybir.AluOpType

    nc = bacc.Bacc("TRN2", target_bir_lowering=False, debug=False,
                   num_devices=NCORES)

    # ---------------- DRAM tensors ----------------
    tok = nc.dram_tensor("tok", [128, 16], i32, kind="ExternalInput").ap()
    embed = nc.dram_tensor("embed", [VOCAB, EDIM], f32, kind="ExternalInput").ap()
    # whh: rhs for recurrence matmuls: [dir, kh, 128, 1024] (cols gate-remapped)
    whh = nc.dram_tensor("whh", [2, 2, 128, 1024], f32, kind="ExternalInput").ap()
    # wih: lhsT tiles for the input projection: [dir, ke, 128, 1024]
    wih = nc.dram_tensor("wih", [2, 2, 128, 1024], f32, kind="ExternalInput").ap()
    # biasT: per (dir, mtile=(g,hi)) per-partition column: [128, 2, 8]
    biasT = nc.dram_tensor("biasT", [128, 2, 8], f32, kind="ExternalInput").ap()
    # wout lhsT tiles: [ks=(hi,dir), 128, 32]
    wout = nc.dram_tensor("wout", [4, 128, 32], f32, kind="ExternalInput").ap()
    boutc = nc.dram_tensor("boutc", [32, 1], f32, kind="ExternalInput").ap()
    idm32 = nc.dram_tensor("idm32", [128, 32], f32, kind="ExternalInput").ap()
    idm128 = nc.dram_tensor("idm128", [128, 128], f32, kind="ExternalInput").ap()
    # DRAM staging for XpT: [dir, m, tc, 128, 512]  (m=(g,hi), tc: 4 chunks of 64 t)
    xpd = nc.dram_tensor("xpd", [2, 8, 4, 128, 512], f32, kind="Internal").ap()
    featd = nc.dram_tensor("feats", [BL, TAGS, S], f32, kind="ExternalOutput").ap()

    TCH = 64   # t-chunk for xp staging
    NTC = S // TCH

    with tile.TileContext(nc) as tc:
        # ---------------- SBUF ----------------
        sb = nc.alloc_sbuf_tensor
        idx_sb = sb("idx_sb", [128, 16], i32).ap()
        x_sb = sb("x_sb", [128, 16, 256], f32).ap()
        xT = sb("xT", [128, 2, 256, 8], f32).ap()          # [u, ke, t, b]
        whh_sb = sb("whh_sb", [128, 2, 2, 1024], f32).ap()  # [u, dir, kh, col]
        wih_sb = sb("wih_sb", [128, 2, 2, 1024], f32).ap()
        biasT_sb = sb("biasT_sb", [128, 2, 8], f32).ap()
        wout_sb = sb("wout_sb", [128, 4, 32], f32).ap()
        bout_sb = sb("bout_sb", [32, 1], f32).ap()
        id32_sb = sb("id32_sb", [128, 32], f32).ap()
        id128_sb = sb("id128_sb", [128, 128], f32).ap()
        xpstage = sb("xpstage", [128, 2, 512], f32).ap()    # double buffer
        xp_buf = sb("xp_buf", [128, 2, 8, 2, TCH, 8], f32).ap()  # [u,(buf),(m),(dir),tl,b]
        zx_sb = sb("zx_sb", [128, 2, 512], f32).ap()        # [u, buf, col] post-psum copy
        zfull = sb("zfull", [128, 8, 2, 8], f32).ap()       # [u, gh, dir, b]
        sg = sb("sg", [128, 6, 2, 8], f32).ap()             # sigmoid(i,f,o)
        tg = sb("tg", [128, 2, 2, 8], f32).ap()             # tanh(g)
        t1 = sb("t1", [128, 2, 2, 8], f32).ap()
        t2 = sb("t2", [128, 2, 2, 8], f32).ap()
        c_st = sb("c_st", [128, 2, 2, 8], f32).ap()
        tc_st = sb("tc_st", [128, 2, 2, 8], f32).ap()
        hz = sb("hz", [128, 2, 2, 8], f32).ap()             # zero h/c init
        hT = sb("hT", [128, 2, 2, S, 8], f32).ap()          # [u, hi, dir, t, b]
        feats_sb = sb("feats_sb", [32, BL, 256], f32).ap()

        # ---------------- PSUM ----------------
        # 6 full banks: zb x2 (double-buffered z), 4 transpose targets (one
        # bank each -- concurrent row-group PE writes to one bank wedge the
        # device). Projection/x-transpose/feats phases reuse these banks.
        ps = nc.alloc_psum_tensor
        zb_ps = [ps(f"zb_ps{i}", [128, 512], f32).ap() for i in range(2)]
        zTg_raw = [ps(f"zTg{i}", [128, 512], f32).ap() for i in range(4)]
        # view [128, (q4, b8)] of each bank; q = (gate-pair, hi) within nh
        zTg = [t[:, 0:32].rearrange("p (q b) -> p q b", q=4) for t in zTg_raw]
        pj_ps = [zb_ps[0], zb_ps[1]]
        xt_ps = zTg_raw[0][:, 0:128]
        f_ps = zTg_raw[1][0:32, 0:256]

        # ---------------- P0: loads + gather ----------------
        nc.vector.memset(hz[:, :, :, :], 0.0)
        nc.vector.memset(zb_ps[0][:, :], 0.0)
        nc.vector.memset(zb_ps[1][:, :], 0.0)
        nc.sync.dma_start(idx_sb[:, :], tok[:, :])
        nc.sync.dma_start(whh_sb[:, :, :, :], whh.rearrange("d k p c -> p d k c"))
        nc.sync.dma_start(wih_sb[:, :, :, :], wih.rearrange("d k p c -> p d k c"))
        nc.sync.dma_start(biasT_sb[:, :, :], biasT)
        nc.sync.dma_start(wout_sb[:, :, :], wout.rearrange("s p n -> p s n"))
        nc.sync.dma_start(bout_sb[:, :], boutc)
        nc.sync.dma_start(id32_sb[:, :], idm32)
        nc.sync.dma_start(id128_sb[:, :], idm128)
        for r in range(16):
            nc.gpsimd.indirect_dma_start(
                out=x_sb[:, r, :], out_offset=None,
                in_=embed[:, :],
                in_offset=bass.IndirectOffsetOnAxis(ap=idx_sb[:, r:r + 1], axis=0),
            )
        tc.strict_bb_all_engine_barrier()

        # ---------------- P1: transpose x -> xT ----------------
        for r in range(16):
            for ke in range(2):
                nc.tensor.transpose(xt_ps[:, :], x_sb[:, r, 128 * ke:128 * (ke + 1)],
                                    id128_sb[:, :])
                nc.vector.tensor_copy(xT[:, ke, 16 * r:16 * (r + 1), :]
                                      .rearrange("p t b -> p (t b)"), xt_ps[:, :])

        # ---------------- P2: input projection -> xpd ----------------
        for d in range(2):
            for m in range(8):
                for c in range(NTC):
                    pp = pj_ps[c % 2]
                    st = xpstage[:, c % 2, :]
                    for ke in range(2):
                        nc.tensor.matmul(
                            pp[:, :],
                            lhsT=wih_sb[:, d, ke, 128 * m:128 * (m + 1)],
                            rhs=xT[:, ke, TCH * c:TCH * (c + 1), :]
                                .rearrange("p t b -> p (t b)"),
                            start=(ke == 0), stop=(ke == 1))
                    nc.vector.tensor_scalar_add(st, pp[:, :],
                                                biasT_sb[:, d, m:m + 1])
                    nc.sync.dma_start(xpd[d, m, c, :, :], st)

        # ---------------- P3: recurrence ----------------
        def lh(kh, d, t_prev):
            if t_prev is None:
                return hz[:, kh, d, :]
            return hT[:, kh, d, t_prev, :]

        # prefetch chunk 0
        def fetch_chunk(cbuf, tcf, tcb):
            # fwd needs t in [tcf*TCH, ...), bwd needs its own chunk
            for d, tcx in ((0, tcf), (1, tcb)):
                for m in range(8):
                    nc.sync.dma_start(
                        xp_buf[:, cbuf, m, d, :, :].rearrange("p t b -> p (t b)"),
                        xpd[d, m, tcx, :, :])

        fetch_chunk(0, 0, NTC - 1)
        tc.strict_bb_all_engine_barrier()
        for k in range(S):
            tf, tb = k, S - 1 - k
            kc = k % TCH
            if kc == 0 and k + TCH < S:
                nxt = (k // TCH + 1) % 2
                fetch_chunk(nxt, k // TCH + 1, NTC - 2 - k // TCH)
            cb = (k // TCH) % 2
            zb = zb_ps[k % 2]
            # matmuls: 4 col groups = (dir, nh)
            for d in range(2):
                t = tf if d == 0 else tb
                tp = (t - 1) if d == 0 else (t + 1)
                if (d == 0 and t == 0) or (d == 1 and t == S - 1):
                    tp = None
                for nh in range(2):
                    g4 = 2 * d + nh
                    for kh in range(2):
                        nc.tensor.matmul(
                            zb[32 * g4:32 * g4 + 8, :],
                            lhsT=lh(kh, d, tp),
                            rhs=whh_sb[:, d, kh, 512 * nh:512 * (nh + 1)],
                            start=(kh == 0), stop=(kh == 1),
                            tile_position=(0, 32 * g4))
            # copy psum -> sbuf (one wide op covers all 4 groups)
            nc.scalar.copy(zx_sb[:, k % 2, :][0:104, :], zb[0:104, :])
            # transposes: z chunks -> zTg psum (one bank per row group)
            for d in range(2):
                for nh in range(2):
                    g4 = 2 * d + nh
                    for q in range(4):
                        nc.tensor.transpose(
                            zTg[g4][:, q, :],
                            zx_sb[:, k % 2, :][32 * g4:32 * g4 + 8,
                                               128 * q:128 * (q + 1)],
                            id32_sb[32 * g4:32 * g4 + 8, 0:8],
                            tile_position=(32 * g4, 0))
            # zfull = zT + xp (bwd chunk is stored t-ascending: reversed index)
            for d in range(2):
                kcd = kc if d == 0 else TCH - 1 - kc
                for nh in range(2):
                    g4 = 2 * d + nh
                    nc.vector.tensor_tensor(
                        out=zfull[:, 4 * nh:4 * (nh + 1), d, :],
                        in0=zTg[g4][:, :, :],
                        in1=xp_buf[:, cb, 4 * nh:4 * (nh + 1), d, kcd, :],
                        op=OP.add)
            # activations
            nc.scalar.activation(sg[:, :, :, :], zfull[:, 0:6, :, :], AF.Sigmoid)
            nc.scalar.activation(tg[:, :, :, :], zfull[:, 6:8, :, :], AF.Tanh)
            # gate math
            nc.vector.tensor_tensor(out=t1[:], in0=sg[:, 0:2, :, :], in1=tg[:],
                                    op=OP.mult)
            first = (k == 0)
            cin = hz if first else c_st
            nc.vector.tensor_tensor(out=t2[:], in0=sg[:, 2:4, :, :], in1=cin[:],
                                    op=OP.mult)
            nc.vector.tensor_tensor(out=c_st[:], in0=t1[:], in1=t2[:], op=OP.add)
            nc.scalar.activation(tc_st[:], c_st[:], AF.Tanh)
            # h -> hT[:, :, :, t, :]  but fwd/bwd write different t!
            for d in range(2):
                t = tf if d == 0 else tb
                nc.vector.tensor_tensor(
                    out=hT[:, :, d, t, :],
                    in0=sg[:, 4:6, d, :], in1=tc_st[:, :, d, :], op=OP.mult)

        # ---------------- P4: feats ----------------
        tc.strict_bb_all_engine_barrier()
        for b in range(BL):
            for ks in range(4):
                hi, d = ks // 2, ks % 2
                nc.tensor.matmul(
                    f_ps[:, :],
                    lhsT=wout_sb[:, ks, :],
                    rhs=hT[:, hi, d, :, b],
                    start=(ks == 0), stop=(ks == 3))
            nc.vector.tensor_scalar_add(feats_sb[:, b, :], f_ps[:, :],
                                        bout_sb[:, 0:1])
        nc.sync.dma_start(featd.rearrange("b n t -> n b t"), feats_sb[:, :, :])

    nc.compile()
    return nc


def _prep_shared(embed, Wih_f, Whh_f, b_f, Wih_b, Whh_b, b_b, Wout, bout):
    """Host-side weight reshapes (replicated to all cores)."""
    gmap = [0, 1, 3, 2]  # new gate order (i,f,o,g) -> orig quarter

    def remap_cols(WT):  # WT [k, 1024-orig] -> [k, 1024-new]
        cols = np.concatenate([WT[:, g * 256:(g + 1) * 256] for g in gmap], axis=1)
        return np.ascontiguousarray(cols)

    def remap_vec(v):
        return np.concatenate([v[g * 256:(g + 1) * 256] for g in gmap])

    whh = np.stack([
        remap_cols(Whh_f.T).reshape(2, 128, 1024),
        remap_cols(Whh_b.T).reshape(2, 128, 1024)]).astype(np.float32)
    wih = np.stack([
        remap_cols(Wih_f.T).reshape(2, 128, 1024),
        remap_cols(Wih_b.T).reshape(2, 128, 1024)]).astype(np.float32)
    # biasT[u, dir, m=(g,hi)] = b_dir[new4h(m*128+u)]
    biasT = np.zeros((128, 2, 8), np.float32)
    for d, bv in enumerate([b_f, b_b]):
        bn = remap_vec(bv)
        biasT[:, d, :] = bn.reshape(8, 128).T
    # wout[ks=(hi,dir), u, n] = Wout[n, dir*256+hi*128+u]
    wout = np.zeros((4, 128, 32), np.float32)
    for ks in range(4):
        hi, d = ks // 2, ks % 2
        wout[ks] = Wout[:, d * 256 + hi * 128:d * 256 + (hi + 1) * 128].T
    boutc = bout.reshape(32, 1).astype(np.float32)
    idm32 = np.zeros((128, 32), np.float32)
    idm32[np.arange(128), np.arange(128) % 32] = 1.0
    idm128 = np.eye(128, dtype=np.float32)
    return dict(embed=np.ascontiguousarray(embed, np.float32), whh=whh, wih=wih,
                biasT=biasT, wout=wout, boutc=boutc, idm32=idm32, idm128=idm128)


def _viterbi_host(feats, trans):
    Bf = feats.shape[0]
    v = np.full((Bf, TAGS), NEG, np.float32)
    v[:, 0] = 0.0
    bps = np.zeros((S, Bf, TAGS), np.int64)
    for t in range(S):
        sc = v[:, :, None] + trans[None]          # [B, prev, next]
        bps[t] = np.argmax(sc, axis=1)
        v = sc.max(axis=1) + feats[:, t, :]
    term = v + trans[0][None]
    scores = term.max(axis=1)
    last = np.argmax(term, axis=1)
    paths = np.zeros((Bf, S), np.int32)
    paths[:, S - 1] = last
    cur = last
    bidx = np.arange(Bf)
    for k in range(S - 2, -1, -1):
        cur = bps[k + 1][bidx, cur]
        paths[:, k] = cur.astype(np.int32)
    return paths, scores.astype(np.float32)


def kernel(sentence, embed, Wih_f, Whh_f, b_f, Wih_b, Whh_b, b_b, Wout, bout,
           trans):
    sentence = np.asarray(sentence)
    trans = np.asarray(trans, np.float32)
    if "nc" not in _CACHE:
        _CACHE["nc"] = _build_bass()
    nc = _CACHE["nc"]

    shared = _prep_shared(np.asarray(embed), np.asarray(Wih_f), np.asarray(Whh_f),
                          np.asarray(b_f), np.asarray(Wih_b), np.asarray(Whh_b),
                          np.asarray(b_b), np.asarray(Wout), np.asarray(bout))
    in_maps = []
    for c in range(NCORES):
        sl = sentence[c * BL:(c + 1) * BL]          # [8, 256]
        tokc = np.ascontiguousarray(
            sl.T.reshape(S * BL).reshape(16, 128).T, np.int32)  # [128, 16]
        m = dict(shared)
        m["tok"] = tokc
        in_maps.append(m)

    from concourse import bass_utils
    res = bass_utils.run_bass_kernel_spmd(nc, in_maps, core_ids=list(range(NCORES)))
    _CACHE["last_result"] = res

    feats_all = np.concatenate(
        [r["feats"] for r in res.results], axis=0)       # [64, 32, 256]
    feats = np.ascontiguousarray(np.swapaxes(feats_all, 1, 2))  # [64, 256(t), 32]
    paths, scores = _viterbi_host(feats, trans)
    return paths, scores
